# revision 1
# baseline (speedup 1.0000x reference)
"""Trainium2 Bass kernel for LoRA causal self-attention (GPT-style block).

Problem: B=4, T=2048, C=1024, H=16 heads, d=64, LoRA rank 8.
reference returns (out, query, key) where
  qkv  = x @ Wa^T + ba + (x @ Aa^T) @ Ba^T
  att  = causal softmax(q k^T / sqrt(d))
  y    = att @ v
  out  = y @ Wp^T + bp + (y @ Ap^T) @ Bp^T

Sharding: 8 cores = (batch b in 0..3) x (head-group g in 0..1, 8 heads each).
Per core the device computes, for its (b, g):
  - qT,kT = (Wqk_eff @ x_b^T) + bias    [feature-major, 512+512 x 2048]
  - v     = x_b @ Wv_eff^T + bias       [token-major, 2048 x 512]
  - per head: S^T = k q^T, exp(S/8) with causal min-mask, y^T via v|1-augmented
    matmul accumulating over key blocks (row 64 of the psum = softmax sums)
  - out^T partial = sum_j wp[j] yT[j] over this core's 512 y-features
LoRA is folded into the weights host-side (exact here since the B matrices are
zero), qkv biases are applied on device, proj bias host-side. The host
transposes/concats per-core outputs and sums the two partial out products per
batch.

All matmuls run as float32r (fp32-width operands streamed at bf16 rate for
moving dim >= 256 on trn2, ~1.5e-4 matmul rel err); tiles feeding matmuls are
float32r end-to-end because walrus requires f32r operands to be *produced*
as f32r. The causal mask is a post-exp 0/1 multiply on the vector engine;
softmax denominators come from a ones-column appended to v (psum row 64).

`reps` replicates the whole compute body inside one NEFF; used by test.py to
measure per-iteration device time as a slope (the axon dispatch overhead is
~60-90 ms, far above the kernel's device time).
"""

import numpy as np

B, T, C, H, D = 4, 2048, 1024, 16, 64
NCORES = 8
HPC = H // 2          # heads per core (head-group of 8)
GF = HPC * D          # features per head-group = 512
TQ = 512              # query tile
KBLK = 128            # key block
XCH = 512             # x token chunk for stage A
_CACHE = {}


def _legalize_waits(nc, mybir):
    """This walrus build rejects any instruction with >1 sync wait; hoist
    extra waits onto single-wait NoOps on the same engine."""
    for fn in nc.m.functions:
        for blk in fn.blocks:
            new_insts = []
            changed = False
            for inst in blk.instructions:
                si = inst.sync_info
                if si is not None and si.on_wait and len(si.on_wait) > 1:
                    for w in si.on_wait:
                        nop = mybir.InstNoOp(
                            name=nc.get_next_instruction_name(),
                            engine=inst.engine,
                            bass_nofuse=True,
                            sync_info=mybir.SyncInfo(on_wait=[w], on_update=[]),
                        )
                        new_insts.append(nop)
                    inst.sync_info = mybir.SyncInfo(
                        on_wait=[], on_update=list(si.on_update)
                    )
                    changed = True
                new_insts.append(inst)
            if changed:
                blk.instructions = new_insts


def _build_nc(reps=1):
    import concourse.bass as bass
    import concourse.mybir as mybir
    import concourse.tile as tile
    from contextlib import ExitStack

    f32 = mybir.dt.float32
    f32r = mybir.dt.float32r
    Exp = mybir.ActivationFunctionType.Exp

    nc = bass.Bass()

    xT = nc.declare_dram_parameter("xT", [C, T], f32r, isOutput=False)
    wqk = nc.declare_dram_parameter("wqk", [C, 2 * GF], f32r, isOutput=False)
    wv = nc.declare_dram_parameter("wv", [C, GF], f32r, isOutput=False)
    wp = nc.declare_dram_parameter("wp", [GF, C], f32r, isOutput=False)
    bqk = nc.declare_dram_parameter("bqk", [128, 8], f32, isOutput=False)
    # multiplicative causal mask for diagonal blocks: m01[p,c] = 1 if c>=p
    m01 = nc.declare_dram_parameter("m01", [128, 128], f32r, isOutput=False)
    q_t = nc.declare_dram_parameter("q_t", [GF, T], f32r, isOutput=True)
    k_t = nc.declare_dram_parameter("k_t", [GF, T], f32r, isOutput=True)
    o_t = nc.declare_dram_parameter("o_t", [C, T], f32, isOutput=True)

    NCH = T // XCH  # x chunks in stage A

    with tile.TileContext(nc) as tc, ExitStack() as ctx:
        p_const = ctx.enter_context(tc.tile_pool(name="const", bufs=1))
        p_yT = ctx.enter_context(tc.tile_pool(name="yT", bufs=1))

        bqk_sb = p_const.tile([128, 8], f32, tag="bqk", name="bqk_sb")
        nc.sync.dma_start(out=bqk_sb[:], in_=bqk[:])
        ones_sb = p_const.tile([128, HPC], f32, tag="ones", name="ones_sb")
        nc.vector.memset(ones_sb[:], 1.0)

        yT_sb = [
            p_yT.tile([128, T], f32r, tag=f"y{j}", name=f"yT{j}") for j in range(4)
        ]

        for _rep in range(reps):
            _emit_body(
                nc, tc, mybir, f32, f32r, Exp,
                xT, wqk, wv, wp, q_t, k_t, o_t,
                m01, bqk_sb, ones_sb, yT_sb, NCH,
            )

    _legalize_waits(nc, mybir)
    return nc


def _emit_body(nc, tc, mybir, f32, f32r, Exp, xT, wqk, wv, wp, q_t, k_t, o_t,
               m01, bqk_sb, ones_sb, yT_sb, NCH):
    with (
        tc.tile_pool(name="qk", bufs=1) as p_qk,
        tc.tile_pool(name="v", bufs=1) as p_v,
    ):
        qk_sb = [
            p_qk.tile([128, T], f32r, tag=f"qk{f}", name=f"qk_sb{f}")
            for f in range(8)
        ]
        # v tiles: per 128-token block, 8 heads x 65 cols; head h occupies
        # cols [65h..65h+64] as [v(64) | 1.0]
        v_sb = [
            p_v.tile([128, HPC * 65], f32r, tag=f"v{i}", name=f"v_sb{i}")
            for i in range(T // KBLK)
        ]

        # ---------------- stage A: input projections ----------------
        with (
            tc.tile_pool(name="w", bufs=1) as p_w,
            tc.tile_pool(name="x", bufs=2) as p_x,
            tc.tile_pool(name="psA", bufs=2, space="PSUM") as psA,
        ):
            wqk_sb = [
                p_w.tile([128, 2 * GF], f32r, tag=f"wqk{c}", name=f"wqk_sb{c}")
                for c in range(8)
            ]
            wv_sb = [
                p_w.tile([128, GF], f32r, tag=f"wv{c}", name=f"wv_sb{c}")
                for c in range(8)
            ]
            x_chunks = {}

            def load_x(ch):
                t0 = ch * XCH
                xs = []
                for c in range(8):
                    xt = p_x.tile([128, XCH], f32r, tag=f"x{c}", name=f"xt{c}",
                                  bufs=(1 if c == 7 else 2))
                    nc.sync.dma_start(
                        out=xt[:], in_=xT[c * 128:(c + 1) * 128, t0:t0 + XCH]
                    )
                    xs.append(xt)
                x_chunks[ch] = xs

            # x chunk 0 first so the first matmul isn't gated on 6 MB of
            # weight DMAs; weights follow on the same queues
            load_x(0)
            for c in range(8):
                nc.sync.dma_start(
                    out=wqk_sb[c][:], in_=wqk[c * 128:(c + 1) * 128, :]
                )
            for c in range(8):
                nc.sync.dma_start(
                    out=wv_sb[c][:], in_=wv[c * 128:(c + 1) * 128, :]
                )

            for ch in range(NCH):
                if ch not in x_chunks:
                    load_x(ch)
                xs = x_chunks.pop(ch)
                t0 = ch * XCH
                # q,k features (feature-major): psum[f-tile, tok]
                for f in range(8):
                    ps = psA.tile([128, XCH], f32, tag="qkps", name="qkps")
                    for c in range(8):
                        nc.tensor.matmul(
                            ps[:],
                            wqk_sb[c][:, f * 128:(f + 1) * 128],
                            xs[c][:],
                            start=(c == 0),
                            stop=(c == 7),
                        )
                    nc.scalar.activation(
                        out=qk_sb[f][:, t0:t0 + XCH],
                        in_=ps[:],
                        func=mybir.ActivationFunctionType.Identity,
                        bias=bqk_sb[:, f:f + 1],
                        scale=1.0,
                    )
                # v (token-major): psum[tok-subtile, feat]
                for sub in range(XCH // KBLK):
                    pv = psA.tile([128, GF], f32, tag="vps", name="vps")
                    for c in range(8):
                        nc.tensor.matmul(
                            pv[:],
                            xs[c][:, sub * 128:(sub + 1) * 128],
                            wv_sb[c][:],
                            start=(c == 0),
                            stop=(c == 7),
                        )
                    ti = ch * (XCH // KBLK) + sub
                    vt = v_sb[ti].rearrange("p (h e) -> p h e", e=65)
                    pvv = pv.rearrange("p (h e) -> p h e", e=64)
                    nc.vector.tensor_copy(out=vt[:, :, 0:64], in_=pvv[:])
                    nc.vector.tensor_copy(
                        out=vt[:, :, 64:65],
                        in_=ones_sb.rearrange("p (h e) -> p h e", e=1),
                    )

        # write q,k outputs (feature-major; host transposes)
        for f in range(4):
            nc.sync.dma_start(out=q_t[f * 128:(f + 1) * 128, :], in_=qk_sb[f][:])
            nc.sync.dma_start(out=k_t[f * 128:(f + 1) * 128, :], in_=qk_sb[4 + f][:])

        # ---------------- stage B: attention (software-pipelined) ----------
        with tc.tile_pool(name="wp", bufs=1) as p_wp:
            # prefetch the output-projection weights during stage B
            wp_sb = [
                p_wp.tile([128, C], f32r, tag=f"wp{j}", name=f"wp_sb{j}")
                for j in range(4)
            ]
            for j in range(4):
                nc.sync.dma_start(out=wp_sb[j][:], in_=wp[j * 128:(j + 1) * 128, :])
            _stage_b(nc, tc, mybir, f32, f32r, Exp, qk_sb, v_sb, yT_sb, m01)
            _stage_c(nc, tc, mybir, f32, f32r, wp_sb, yT_sb, o_t)


def _stage_b(nc, tc, mybir, f32, f32r, Exp, qk_sb, v_sb, yT_sb, m01):
        with (
            tc.tile_pool(name="att", bufs=3) as p_att,
            tc.tile_pool(name="sm", bufs=2) as p_sm,
            tc.tile_pool(name="dscr", bufs=2, space="DRAM") as p_dscr,
            tc.tile_pool(name="psS", bufs=2, space="PSUM") as psS,
            tc.tile_pool(name="psY", bufs=2, space="PSUM") as psY,
        ):
            m01_sb = p_sm.tile([128, 128], f32r, tag="m01", name="m01_sb", bufs=1)
            nc.sync.dma_start(out=m01_sb[:], in_=m01[:])

            blocks = [
                (hp, qt, kb)
                for hp in range(4)
                for qt in range(4)
                for kb in range(4 * qt + 4)
            ]
            tiles = {}   # block idx -> (sps, att)
            ytiles = {}  # (hp, qt) -> [yps0, yps1]

            def emit_s(i):
                hp, qt, kb = blocks[i]
                j = kb - 4 * qt
                col0 = max(0, j) * 128
                qtile = qk_sb[hp]
                ktile = qk_sb[4 + hp]
                # both heads share one 2-bank psum / att tile:
                # cols [0:512] = head hi=0, [512:1024] = head hi=1
                sps = psS.tile([128, 2 * TQ], f32, tag="s", name="sps")
                att = p_att.tile([128, 2 * TQ], f32r, tag="att", name="att")
                for hi in range(2):
                    row0 = hi * 64
                    c0 = hi * TQ
                    nc.tensor.matmul(
                        sps[:, c0 + col0:c0 + TQ],
                        ktile[row0:row0 + 64, kb * 128:(kb + 1) * 128],
                        qtile[row0:row0 + 64, qt * TQ + col0:(qt + 1) * TQ],
                        start=True,
                        stop=True,
                    )
                tiles[i] = (sps, att)

            def emit_ea(i):
                hp, qt, kb = blocks[i]
                j = kb - 4 * qt
                col0 = max(0, j) * 128
                sps, att = tiles.pop(i)
                if kb == 0:
                    ytiles[(hp, qt)] = [
                        psY.tile([128, TQ], f32, tag=f"y{hi}", name=f"yps{hi}")
                        for hi in range(2)
                    ]
                yps = ytiles[(hp, qt)]
                if j < 0:
                    nc.scalar.activation(
                        out=att[:, 0:2 * TQ], in_=sps[:, 0:2 * TQ],
                        func=Exp, scale=0.125,
                    )
                else:
                    for hi in range(2):
                        c0 = hi * TQ
                        nc.scalar.activation(
                            out=att[:, c0 + col0:c0 + TQ],
                            in_=sps[:, c0 + col0:c0 + TQ],
                            func=Exp, scale=0.125,
                        )
                        # zero the causally-invalid lower triangle
                        nc.vector.tensor_mul(
                            out=att[:, c0 + col0:c0 + col0 + 128],
                            in0=att[:, c0 + col0:c0 + col0 + 128],
                            in1=m01_sb[:],
                        )
                nkb = 4 * qt + 4
                for hi in range(2):
                    c0 = hi * TQ
                    h = 2 * hp + hi
                    v65 = v_sb[kb][:, h * 65:h * 65 + 65]
                    # psum rows 0..63 = y^T, row 64 = softmax sum
                    nc.tensor.matmul(
                        yps[hi][0:65, col0:TQ],
                        v65,
                        att[:, c0 + col0:c0 + TQ],
                        start=(kb == 0),
                        stop=(kb == nkb - 1),
                    )
                if kb == nkb - 1:
                    emit_evac(hp, qt)

            def emit_evac(hp, qt):
                yps = ytiles.pop((hp, qt))
                for hi in range(2):
                    ysrc = yps[hi]
                    rec = p_sm.tile([128, TQ], f32, tag="rec", name="rec")
                    nc.vector.reciprocal(out=rec[64:65, :], in_=ysrc[64:65, :])
                    # broadcast row 64 -> rows 0..63 via DRAM bounce
                    # (SBUF->SBUF partition-broadcast DMA is illegal)
                    dscr = p_dscr.tile([1, TQ], f32, tag="dscr", name="dscr")
                    nc.sync.dma_start(out=dscr[:], in_=rec[64:65, :])
                    nc.sync.dma_start(
                        out=rec[0:64, :], in_=dscr[:].to_broadcast([64, TQ])
                    )
                    if hi == 0:
                        nc.vector.tensor_mul(
                            out=yT_sb[hp][0:64, qt * TQ:(qt + 1) * TQ],
                            in0=ysrc[0:64, :],
                            in1=rec[0:64, :],
                        )
                    else:
                        tmp = p_sm.tile([128, TQ], f32r, tag="tmp", name="tmp")
                        nc.vector.tensor_mul(
                            out=tmp[0:64, :],
                            in0=ysrc[0:64, :],
                            in1=rec[0:64, :],
                        )
                        nc.sync.dma_start(
                            out=yT_sb[hp][64:128, qt * TQ:(qt + 1) * TQ],
                            in_=tmp[0:64, :],
                        )

            for i in range(len(blocks) + 1):
                if i < len(blocks):
                    emit_s(i)
                if i >= 1:
                    emit_ea(i - 1)


def _stage_c(nc, tc, mybir, f32, f32r, wp_sb, yT_sb, o_t):
    with (
        tc.tile_pool(name="og", bufs=3) as p_og,
        tc.tile_pool(name="psC", bufs=3, space="PSUM") as psC,
    ):
        for ot in range(8):
            for tch in range(4):
                pp = psC.tile([128, TQ], f32, tag="pp", name="pp")
                for j in range(4):
                    nc.tensor.matmul(
                        pp[:],
                        wp_sb[j][:, ot * 128:(ot + 1) * 128],
                        yT_sb[j][:, tch * TQ:(tch + 1) * TQ],
                        start=(j == 0),
                        stop=(j == 3),
                    )
                og = p_og.tile([128, TQ], f32, tag="og", name="og")
                nc.vector.tensor_copy(out=og[:], in_=pp[:])
                nc.sync.dma_start(
                    out=o_t[ot * 128:(ot + 1) * 128, tch * TQ:(tch + 1) * TQ],
                    in_=og[:],
                )


def get_nc(reps=1):
    key = f"nc{reps}"
    if key not in _CACHE:
        _CACHE[key] = _build_nc(reps)
    return _CACHE[key]


def make_in_maps(x, Wa_eff, ba, Wp_eff):
    """Build the 8 per-core input maps from full tensors."""
    m01 = (np.arange(128)[None, :] >= np.arange(128)[:, None]).astype(np.float32)
    in_maps = []
    for core in range(NCORES):
        b, g = core // 2, core % 2
        sl = slice(g * GF, (g + 1) * GF)
        wq = Wa_eff[0:C][sl]
        wk = Wa_eff[C:2 * C][sl]
        wvm = Wa_eff[2 * C:3 * C][sl]
        bq = ba[0:C][sl]
        bk = ba[C:2 * C][sl]
        bvv = ba[2 * C:3 * C][sl]
        in_maps.append({
            "xT": np.ascontiguousarray(x[b].T),
            "wqk": np.ascontiguousarray(np.concatenate([wq, wk], axis=0).T),
            "wv": np.ascontiguousarray(wvm.T),
            "wp": np.ascontiguousarray(Wp_eff[:, sl].T),
            "bqk": np.ascontiguousarray(
                np.concatenate([bq, bk]).reshape(8, 128).T
            ),
            "bv": np.ascontiguousarray(bvv),
            "m01": m01,
        })
    return in_maps


def assemble(results, bp):
    """Combine per-core outputs into (out, query, key)."""
    query = np.empty((B, T, C), np.float32)
    key = np.empty((B, T, C), np.float32)
    out = np.zeros((B, T, C), np.float32)
    for core in range(NCORES):
        b, g = core // 2, core % 2
        sl = slice(g * GF, (g + 1) * GF)
        r = results[core]
        query[b, :, sl] = r["q_t"].T
        key[b, :, sl] = r["k_t"].T
        out[b] += r["o_t"].T
    out += bp[None, None, :]
    return out, query, key


def kernel(**inputs):
    from concourse.bass_utils import run_bass_kernel_spmd

    x = np.asarray(inputs["x"], np.float32)
    Wa = np.asarray(inputs["c_attn_w"], np.float32)
    ba = np.asarray(inputs["c_attn_b"], np.float32)
    Aa = np.asarray(inputs["c_attn_A"], np.float32)
    Ba = np.asarray(inputs["c_attn_B"], np.float32)
    Wp = np.asarray(inputs["c_proj_w"], np.float32)
    bp = np.asarray(inputs["c_proj_b"], np.float32)
    Ap = np.asarray(inputs["c_proj_A"], np.float32)
    Bp = np.asarray(inputs["c_proj_B"], np.float32)
    n_head = int(np.asarray(inputs["n_head"]))
    assert n_head == H and x.shape == (B, T, C)

    Wa_eff = (Wa + Ba.astype(np.float64) @ Aa.astype(np.float64)).astype(np.float32)
    Wp_eff = (Wp + Bp.astype(np.float64) @ Ap.astype(np.float64)).astype(np.float32)

    nc = get_nc()
    in_maps = make_in_maps(x, Wa_eff, ba, Wp_eff)
    res = run_bass_kernel_spmd(nc, in_maps, core_ids=list(range(NCORES)))
    return assemble(res.results, bp)



# revision 4
# speedup vs baseline: 6.0976x; 6.0976x over previous
"""Trainium2 Bass kernel v3 for LoRA causal self-attention (GPT-style block).

Sharding as kernel.py: 8 cores = (batch b) x (head-group g of 8 heads).
All-bf16 operand pipeline (PSUM f32), restructured for single-execution
latency:

  phase 0: load x chunk0 (tokens 0..1023) + weights, QKV projections for
           chunk 0 (1024-wide moving matmuls).
  phase 1: stage-A chunk 1 interleaved with attention for q-tile 0
           (q 0..1023, all 8 heads) so the PE fills scalar-engine (exp)
           gaps with projection work. PSUM: 3 banks stage A + 2 psS +
           2 psY.
  phase 2: attention q-tile 1 (q 1024..2047, kb 0..15 per head),
           double-buffered psS/psY (8 banks).
  phase 3: output projection, 1024-wide moving, token-chunk 0 first.

Attention runs per head on [128 keys, 1024 q] blocks: S^T = k q^T in psum,
exp via scalar engine to bf16 att, causal triangle masked by a 0/1
multiply, y^T accumulated via the v|1-augmented matmul (psum row 64 =
softmax sums). Denominators are broadcast across partitions with a
ones-stationary matmul (rps = ones^T @ recip-row) instead of a DRAM
round trip; the normalize multiply writes odd heads directly at
partition offset 64. Matmul PSUM writes never cross a 2KB bank (512
f32 cols) and DVE instructions read at most one PSUM operand — both
hard walrus/HW constraints.
"""

import numpy as np

B, T, C, H, D = 4, 2048, 1024, 16, 64
NCORES = 8
HPC = H // 2          # heads per core
GF = HPC * D          # features per head-group = 512
TQB = 1024            # per-head q tile
KBLK = 128            # key block
XCH = 1024            # x token chunk for stage A
_CACHE = {}


def _legalize_waits(nc, mybir):
    """This walrus build rejects any instruction with >1 sync wait; hoist
    extra waits onto single-wait NoOps on the same engine."""
    for fn in nc.m.functions:
        for blk in fn.blocks:
            new_insts = []
            changed = False
            for inst in blk.instructions:
                si = inst.sync_info
                if si is not None and si.on_wait and len(si.on_wait) > 1:
                    for w in si.on_wait:
                        nop = mybir.InstNoOp(
                            name=nc.get_next_instruction_name(),
                            engine=inst.engine,
                            bass_nofuse=True,
                            sync_info=mybir.SyncInfo(on_wait=[w], on_update=[]),
                        )
                        new_insts.append(nop)
                    inst.sync_info = mybir.SyncInfo(
                        on_wait=[], on_update=list(si.on_update)
                    )
                    changed = True
                new_insts.append(inst)
            if changed:
                blk.instructions = new_insts


def _build_nc(reps=1):
    import concourse.bass as bass
    import concourse.mybir as mybir
    import concourse.tile as tile
    from contextlib import ExitStack

    f32 = mybir.dt.float32
    bf16 = mybir.dt.bfloat16

    nc = bass.Bass()

    xT = nc.declare_dram_parameter("xT", [C, T], bf16, isOutput=False)
    wqk = nc.declare_dram_parameter("wqk", [C, 2 * GF], bf16, isOutput=False)
    wv = nc.declare_dram_parameter("wv", [C, GF], bf16, isOutput=False)
    wp = nc.declare_dram_parameter("wp", [GF, C], bf16, isOutput=False)
    bqk = nc.declare_dram_parameter("bqk", [128, 8], f32, isOutput=False)
    m01 = nc.declare_dram_parameter("m01", [128, 128], bf16, isOutput=False)
    q_t = nc.declare_dram_parameter("q_t", [GF, T], bf16, isOutput=True)
    k_t = nc.declare_dram_parameter("k_t", [GF, T], bf16, isOutput=True)
    o_t = nc.declare_dram_parameter("o_t", [C, T], bf16, isOutput=True)

    with tile.TileContext(nc) as tc, ExitStack() as ctx:
        p_const = ctx.enter_context(tc.tile_pool(name="const", bufs=1))
        p_qk = ctx.enter_context(tc.tile_pool(name="qk", bufs=1))
        p_v = ctx.enter_context(tc.tile_pool(name="v", bufs=1))
        p_yT = ctx.enter_context(tc.tile_pool(name="yT", bufs=1))

        bqk_sb = p_const.tile([128, 8], f32, tag="bqk", name="bqk_sb")
        nc.sync.dma_start(out=bqk_sb[:], in_=bqk[:])
        m01_sb = p_const.tile([128, 128], bf16, tag="m01", name="m01_sb")
        nc.sync.dma_start(out=m01_sb[:], in_=m01[:])
        ones_row = p_const.tile([1, 128], bf16, tag="onr", name="ones_row")
        nc.vector.memset(ones_row[:], 1.0)

        qk_sb = [
            p_qk.tile([128, T], bf16, tag=f"qk{f}", name=f"qk_sb{f}")
            for f in range(8)
        ]
        # v tiles: per 128-token block, 8 heads x 65 cols; head h occupies
        # cols [65h..65h+64] as [v(64) | 1.0]. The ones column is constant:
        # preset the whole tile once, data copies overwrite cols 0..63.
        v_sb = [
            p_v.tile([128, HPC * 65], bf16, tag=f"v{i}", name=f"v_sb{i}")
            for i in range(T // KBLK)
        ]
        for i in range(T // KBLK):
            nc.vector.memset(v_sb[i][:], 1.0)
        yT_sb = [
            p_yT.tile([128, T], bf16, tag=f"y{j}", name=f"yT{j}") for j in range(4)
        ]

        for _rep in range(reps):
            _emit_body(
                nc, tc, mybir, f32, bf16,
                xT, wqk, wv, wp, q_t, k_t, o_t,
                m01_sb, bqk_sb, ones_row, qk_sb, v_sb, yT_sb,
            )

    _legalize_waits(nc, mybir)
    return nc


def _emit_body(nc, tc, mybir, f32, bf16, xT, wqk, wv, wp, q_t, k_t, o_t,
               m01_sb, bqk_sb, ones_row, qk_sb, v_sb, yT_sb):
    Exp = mybir.ActivationFunctionType.Exp

    with (
        tc.tile_pool(name="w", bufs=1) as p_w,
        tc.tile_pool(name="x", bufs=2) as p_x,
        tc.tile_pool(name="att", bufs=4) as p_att,
        tc.tile_pool(name="pre", bufs=1) as p_pre,
        tc.tile_pool(name="sm", bufs=2) as p_sm,
        tc.tile_pool(name="wp", bufs=1) as p_wp,
    ):
        wqk_sb = [
            p_w.tile([128, 2 * GF], bf16, tag=f"wqk{c}", name=f"wqk_sb{c}")
            for c in range(8)
        ]
        wv_sb = [
            p_w.tile([128, GF], bf16, tag=f"wv{c}", name=f"wv_sb{c}")
            for c in range(8)
        ]
        wp_sb = [
            p_wp.tile([128, C], bf16, tag=f"wp{j}", name=f"wp_sb{j}")
            for j in range(4)
        ]

        def load_x(ch, interleave_w=False):
            t0 = ch * XCH
            xs = []
            for c in range(8):
                xt = p_x.tile([128, XCH], bf16, tag=f"x{c}", name=f"xt{c}",
                              bufs=(1 if c == 7 else 2))
                nc.sync.dma_start(
                    out=xt[:], in_=xT[c * 128:(c + 1) * 128, t0:t0 + XCH]
                )
                if interleave_w:
                    nc.sync.dma_start(
                        out=wqk_sb[c][:], in_=wqk[c * 128:(c + 1) * 128, :]
                    )
                xs.append(xt)
            return xs

        # ---- stage-A emission units -------------------------------------
        def emit_qk(xs, ch, f, half):
            """One qk psum group: features f*128..f*128+127, one 512-token
            half of chunk ch (psum matmul writes must stay in one bank)."""
            t0 = ch * XCH
            cols = half * 512
            ps = psA.tile([128, 512], f32, tag="qkps", name="qkps")
            for c in range(8):
                nc.tensor.matmul(
                    ps[:],
                    wqk_sb[c][:, f * 128:(f + 1) * 128],
                    xs[c][:, cols:cols + 512],
                    start=(c == 0),
                    stop=(c == 7),
                )
            nc.vector.tensor_scalar_add(
                out=qk_sb[f][:, t0 + cols:t0 + cols + 512],
                in0=ps[:],
                scalar1=bqk_sb[:, f:f + 1],
            )

        def emit_v(xs, ch, sub):
            """One v psum group: tokens ch*1024 + sub*128 .. +127."""
            pv = psA.tile([128, GF], f32, tag="vps", name="vps")
            for c in range(8):
                nc.tensor.matmul(
                    pv[:],
                    xs[c][:, sub * 128:(sub + 1) * 128],
                    wv_sb[c][:],
                    start=(c == 0),
                    stop=(c == 7),
                )
            ti = ch * (XCH // KBLK) + sub
            vt = v_sb[ti].rearrange("p (h e) -> p h e", e=65)
            pvv = pv.rearrange("p (h e) -> p h e", e=64)
            nc.vector.tensor_copy(out=vt[:, :, 0:64], in_=pvv[:])

        def make_a_dripper(xs, units):
            """Incremental stage-A emitter: each call emits `n` matmuls of
            chunk-1 projection work, finishing psum groups with their DVE
            evac as they complete. Keeps PE fed during exp-wait gaps without
            parking long accumulation chains in front of stage-B matmuls."""
            state = {"ui": 0, "c": 0, "ps": None}

            def drip(n):
                while n > 0 and state["ui"] < len(units):
                    kind, p1, p2 = units[state["ui"]]
                    if state["ps"] is None:
                        if kind == "qk":
                            w = 512
                            state["ps"] = psA.tile(
                                [128, w], f32, tag="dqk", name="qkps"
                            )
                        else:
                            state["ps"] = psA.tile(
                                [128, GF], f32, tag="vps", name="vps"
                            )
                        state["c"] = 0
                    ps = state["ps"]
                    c0 = state["c"]
                    take = min(2, 8 - c0, n)
                    for c in range(c0, c0 + take):
                        if kind == "qk":
                            cols = p2 * 512
                            nc.tensor.matmul(
                                ps[:],
                                wqk_sb[c][:, p1 * 128:(p1 + 1) * 128],
                                xs[c][:, cols:cols + 512],
                                start=(c == 0),
                                stop=(c == 7),
                            )
                        else:
                            nc.tensor.matmul(
                                ps[:],
                                xs[c][:, p1 * 128:(p1 + 1) * 128],
                                wv_sb[c][:],
                                start=(c == 0),
                                stop=(c == 7),
                            )
                    state["c"] += take
                    n -= take
                    if state["c"] == 8:
                        if kind == "qk":
                            cols = p2 * 512
                            nc.vector.tensor_scalar_add(
                                out=qk_sb[p1][:, XCH + cols:XCH + cols + 512],
                                in0=ps[:],
                                scalar1=bqk_sb[:, p1:p1 + 1],
                            )
                        else:
                            ti = 8 + p1
                            vt = v_sb[ti].rearrange("p (h e) -> p h e", e=65)
                            pvv = ps.rearrange("p (h e) -> p h e", e=64)
                            nc.vector.tensor_copy(out=vt[:, :, 0:64], in_=pvv[:])
                        state["ps"] = None
                        state["ui"] += 1
                return state["ui"] >= len(units)

            return drip

        def emit_qkout(ch):
            t0 = ch * XCH
            for f in range(4):
                nc.sync.dma_start(
                    out=q_t[f * 128:(f + 1) * 128, t0:t0 + XCH],
                    in_=qk_sb[f][:, t0:t0 + XCH],
                )
                nc.sync.dma_start(
                    out=k_t[f * 128:(f + 1) * 128, t0:t0 + XCH],
                    in_=qk_sb[4 + f][:, t0:t0 + XCH],
                )

        # ---- stage-B emission units -------------------------------------
        sblocks = {}  # (h, qt2, kb) -> (sps, att)
        # phase-2 blocks precomputed (S+exp) during phase-1 scalar-engine
        # slack: heads 0..3, kb 0..3 of q-tile 1 (off-diagonal, mask-free)
        PREH, PREK = 4, 4
        pre_att = {}  # (h, kb) -> att tile

        def emit_s(h, qt2, kb):
            f, row0 = h // 2, (h % 2) * 64
            col0 = max(0, kb - 8 * qt2) * 128
            sps = psS.tile([128, TQB], f32, tag="s", name="sps")
            att = p_att.tile([128, TQB], bf16, tag="att", name="att")
            kt = qk_sb[4 + f][row0:row0 + 64, kb * KBLK:(kb + 1) * KBLK]
            for a, b in ((col0, 512), (max(col0, 512), TQB)):
                if a >= b:
                    continue
                nc.tensor.matmul(
                    sps[:, a:b],
                    kt,
                    qk_sb[f][row0:row0 + 64, qt2 * TQB + a:qt2 * TQB + b],
                    start=True,
                    stop=True,
                )
            sblocks[(h, qt2, kb)] = (sps, att)

        def emit_ea(h, qt2, kb, yps, nkb):
            j = kb - 8 * qt2
            col0 = max(0, j) * 128
            sps, att = sblocks.pop((h, qt2, kb))
            nc.scalar.activation(
                out=att[:, col0:TQB], in_=sps[:, col0:TQB],
                func=Exp, scale=0.125,
            )
            if j >= 0:
                nc.vector.tensor_mul(
                    out=att[:, col0:col0 + 128],
                    in0=att[:, col0:col0 + 128],
                    in1=m01_sb[:],
                )
            v65 = v_sb[kb][:, (h * 65):(h * 65 + 65)]
            # half [0:512] is last written at kb = 8*qt2+3 (col0 past it
            # after that); half [512:1024] at the final key block
            for a, b in ((col0, 512), (max(col0, 512), TQB)):
                if a >= b:
                    continue
                last = 8 * qt2 + 3 if b == 512 else nkb - 1
                nc.tensor.matmul(
                    yps[0:65, a:b],
                    v65,
                    att[:, a:b],
                    start=(kb == 0),
                    stop=(kb == last),
                )

        def emit_pre(h, kb):
            """S + exp for (h, qt2=1, kb<8): off-diagonal, full 1024 q cols.
            att lands in a long-lived SBUF tile consumed by phase 2."""
            f, row0 = h // 2, (h % 2) * 64
            att = p_pre.tile([128, TQB], bf16, tag=f"pre{h}_{kb}", name="pre")
            sps = psS.tile([128, TQB], f32, tag="s", name="sps")
            kt = qk_sb[4 + f][row0:row0 + 64, kb * KBLK:(kb + 1) * KBLK]
            for a, b in ((0, 512), (512, TQB)):
                nc.tensor.matmul(
                    sps[:, a:b],
                    kt,
                    qk_sb[f][row0:row0 + 64, TQB + a:TQB + b],
                    start=True,
                    stop=True,
                )
            nc.scalar.activation(
                out=att[:], in_=sps[:], func=Exp, scale=0.125,
            )
            pre_att[(h, kb)] = att

        def emit_y_pre(h, kb, yps):
            """y matmul for a precomputed block (kb<8 of q-tile 1)."""
            att = pre_att.pop((h, kb))
            v65 = v_sb[kb][:, (h * 65):(h * 65 + 65)]
            for a, b in ((0, 512), (512, TQB)):
                last = 11 if b == 512 else 15
                nc.tensor.matmul(
                    yps[0:65, a:b],
                    v65,
                    att[:, a:b],
                    start=(kb == 0),
                    stop=(kb == last),
                )

        def emit_evac(h, qt2, yps):
            hp, hi = h // 2, h % 2
            rec = p_sm.tile([1, TQB], bf16, tag="rec", name="rec")
            with nc.allow_low_precision(reason="softmax denom broadcast in bf16"):
                nc.vector.reciprocal(out=rec[:], in_=yps[64:65, :])
            # broadcast 1/sum across partitions via ones-stationary matmul
            # (psum tile aliases the psS rotation; no DRAM round trip)
            rps = psS.tile([128, TQB], f32, tag="s", name="rps")
            for a in (0, 512):
                nc.tensor.matmul(
                    rps[:, a:a + 512], ones_row[:], rec[:, a:a + 512],
                    start=True, stop=True,
                )
            # walrus allows only one PSUM operand per DVE instruction
            recb = p_sm.tile([64, TQB], bf16, tag="recb", name="recb")
            nc.vector.tensor_copy(out=recb[:], in_=rps[0:64, :])
            nc.vector.tensor_mul(
                out=yT_sb[hp][hi * 64:hi * 64 + 64, qt2 * TQB:(qt2 + 1) * TQB],
                in0=yps[0:64, :],
                in1=recb[:],
            )

        # ================= phase 0: chunk 0 projections ==================
        xs0 = load_x(0, interleave_w=True)
        for c in range(8):
            nc.sync.dma_start(out=wv_sb[c][:], in_=wv[c * 128:(c + 1) * 128, :])
        with tc.tile_pool(name="psA0", bufs=2, space="PSUM") as psA:
            for f in range(8):
                emit_qk(xs0, 0, f, 0)
                emit_qk(xs0, 0, f, 1)
            for sub in range(8):
                emit_v(xs0, 0, sub)
        emit_qkout(0)
        # prefetch output-projection weights behind stage-A traffic
        for j in range(4):
            nc.sync.dma_start(out=wp_sb[j][:], in_=wp[j * 128:(j + 1) * 128, :])

        # ===== phase 1: chunk 1 projections woven with q-tile-0 attn =====
        xs1 = load_x(1)
        # qk half-groups alternate the two psum buffers (tag keyed on half)
        a_units = (
            [("qk", f, hf) for f in range(8) for hf in range(2)]
            + [("v", sub, None) for sub in range(8)]
        )
        with (
            tc.tile_pool(name="psA1", bufs=1, space="PSUM") as psA,
            tc.tile_pool(name="psS1", bufs=2, space="PSUM") as psS,
            tc.tile_pool(name="psY1", bufs=1, space="PSUM") as psY,
        ):
            drip = make_a_dripper(xs1, a_units)
            # 192 chunk-1 matmuls over 64 blocks; bias the budget to late
            # heads (execution outruns the block chain early in the phase).
            # Evacs are deferred one block into the next head so the next
            # head's first S matmul reaches the scalar engine promptly.
            pend = None
            pend_evac = None
            yh = {}
            pre_units = [(h, kb) for h in range(PREH) for kb in range(PREK)]
            pi = 0
            for h in range(8):
                yh[h] = psY.tile([128, TQB], f32, tag="yy", name="yps")
                for kb in range(8):
                    emit_s(h, 0, kb)
                    drip(2 if h < 4 else 4)
                    if pend is not None:
                        emit_ea(*pend)
                    pend = (h, 0, kb, yh[h], 8)
                    if pend_evac is not None and kb == 0:
                        emit_evac(*pend_evac)
                        pend_evac = None
                    # late phase 1: PE and Act both have slack once the
                    # drip budget thins; precompute phase-2 blocks
                    if h >= 5 and pi < len(pre_units):
                        emit_pre(*pre_units[pi])
                        pi += 1
                emit_ea(*pend)
                pend = None
                pend_evac = (h, 0, yh[h])
            emit_evac(*pend_evac)
            drip(len(a_units) * 8)
            while pi < len(pre_units):
                emit_pre(*pre_units[pi])
                pi += 1
        emit_qkout(1)

        # ================= phase 2: q-tile-1 attention ===================
        with (
            tc.tile_pool(name="psS2", bufs=2, space="PSUM") as psS,
            tc.tile_pool(name="psY2", bufs=2, space="PSUM") as psY,
        ):
            pend = None
            pend_evac = None
            yh = {}
            for h in range(8):
                yh[h] = psY.tile([128, TQB], f32, tag="yy", name="yps")
                pre_k = PREK if h < PREH else 0
                if pre_k:
                    # feed the scalar engine first, then drain precomputed y
                    emit_s(h, 1, pre_k)
                    for kb in range(pre_k):
                        emit_y_pre(h, kb, yh[h])
                    pend = (h, 1, pre_k, yh[h], 16)
                    if pend_evac is not None:
                        emit_evac(*pend_evac)
                        pend_evac = None
                for kb in range(pre_k, 16):
                    if not (pre_k and kb == pre_k):
                        emit_s(h, 1, kb)
                        if pend is not None:
                            emit_ea(*pend)
                        pend = (h, 1, kb, yh[h], 16)
                    if pend_evac is not None and kb == 4:
                        emit_evac(*pend_evac)
                        pend_evac = None
                emit_ea(*pend)
                pend = None
                pend_evac = (h, 1, yh[h])
            emit_evac(*pend_evac)

        # ================= phase 3: output projection ====================
        with (
            tc.tile_pool(name="og", bufs=3) as p_og,
            tc.tile_pool(name="psC", bufs=3, space="PSUM") as psC,
        ):
            for tch in range(2):
                for ot in range(8):
                    pp = psC.tile([128, TQB], f32, tag="pp", name="pp")
                    for j in range(4):
                        for a in (0, 512):
                            nc.tensor.matmul(
                                pp[:, a:a + 512],
                                wp_sb[j][:, ot * 128:(ot + 1) * 128],
                                yT_sb[j][:, tch * TQB + a:tch * TQB + a + 512],
                                start=(j == 0),
                                stop=(j == 3),
                            )
                    og = p_og.tile([128, TQB], bf16, tag="og", name="og")
                    nc.vector.tensor_copy(out=og[:], in_=pp[:])
                    nc.sync.dma_start(
                        out=o_t[ot * 128:(ot + 1) * 128, tch * TQB:(tch + 1) * TQB],
                        in_=og[:],
                    )


def get_nc(reps=1):
    key = f"nc{reps}"
    if key not in _CACHE:
        _CACHE[key] = _build_nc(reps)
    return _CACHE[key]


def _bf16(a):
    import ml_dtypes
    return np.ascontiguousarray(a.astype(ml_dtypes.bfloat16))


def make_in_maps(x, Wa_eff, ba, Wp_eff):
    """Build the 8 per-core input maps from full tensors."""
    m01 = (np.arange(128)[None, :] >= np.arange(128)[:, None]).astype(np.float32)
    in_maps = []
    for core in range(NCORES):
        b, g = core // 2, core % 2
        sl = slice(g * GF, (g + 1) * GF)
        wq = Wa_eff[0:C][sl]
        wk = Wa_eff[C:2 * C][sl]
        wvm = Wa_eff[2 * C:3 * C][sl]
        bq = ba[0:C][sl]
        bk = ba[C:2 * C][sl]
        in_maps.append({
            "xT": _bf16(x[b].T),
            "wqk": _bf16(np.concatenate([wq, wk], axis=0).T),
            "wv": _bf16(wvm.T),
            "wp": _bf16(Wp_eff[:, sl].T),
            "bqk": np.ascontiguousarray(
                np.concatenate([bq, bk]).reshape(8, 128).T
            ),
            "m01": _bf16(m01),
        })
    return in_maps


def assemble(results, bp):
    """Combine per-core outputs into (out, query, key)."""
    query = np.empty((B, T, C), np.float32)
    key = np.empty((B, T, C), np.float32)
    out = np.zeros((B, T, C), np.float32)
    for core in range(NCORES):
        b, g = core // 2, core % 2
        sl = slice(g * GF, (g + 1) * GF)
        r = results[core]
        query[b, :, sl] = r["q_t"].astype(np.float32).T
        key[b, :, sl] = r["k_t"].astype(np.float32).T
        out[b] += r["o_t"].astype(np.float32).T
    out += bp[None, None, :]
    return out, query, key


def kernel(**inputs):
    from concourse.bass_utils import run_bass_kernel_spmd

    x = np.asarray(inputs["x"], np.float32)
    Wa = np.asarray(inputs["c_attn_w"], np.float32)
    ba = np.asarray(inputs["c_attn_b"], np.float32)
    Aa = np.asarray(inputs["c_attn_A"], np.float32)
    Ba = np.asarray(inputs["c_attn_B"], np.float32)
    Wp = np.asarray(inputs["c_proj_w"], np.float32)
    bp = np.asarray(inputs["c_proj_b"], np.float32)
    Ap = np.asarray(inputs["c_proj_A"], np.float32)
    Bp = np.asarray(inputs["c_proj_B"], np.float32)
    n_head = int(np.asarray(inputs["n_head"]))
    assert n_head == H and x.shape == (B, T, C)

    Wa_eff = (Wa + Ba.astype(np.float64) @ Aa.astype(np.float64)).astype(np.float32)
    Wp_eff = (Wp + Bp.astype(np.float64) @ Ap.astype(np.float64)).astype(np.float32)

    nc = get_nc()
    in_maps = make_in_maps(x, Wa_eff, ba, Wp_eff)
    res = run_bass_kernel_spmd(nc, in_maps, core_ids=list(range(NCORES)))
    return assemble(res.results, bp)
